# revision 24
# baseline (speedup 1.0000x reference)
"""Multi-head attention (B=2, T=2048, D=1024, H=16) on 8 TRN2 NeuronCores.

Sharding: tensor-parallel over heads — 2 heads per core. Each core computes
QKV for its heads (full token range), attention, and a partial output
projection against its column shard of w_proj; the host sums the 8 partials.

Per-core device program (SPMD, identical program, per-core weight shards):
  inputs (host-prepared, bf16, pre-tiled for contiguous DMA):
    xT  [128, 8, 8, 512]  x flat [B*T, D], transposed (d on partitions) and
                          tiled as [partition, t-chunk, d-tile, t]
    wT  [128, 8, 384]     w_qkv rows for this core's heads, transposed and
                          d-tiled (e cols: q0 q1 | k0 k1 | v0 v1, 64 each)
    wpT [128, 1024]       w_proj columns for this core's heads, transposed
  output (f32):
    y   [4096, 1024]  partial projection output (summed across cores on host)

All matmuls run in bf16 with f32 PSUM accumulation (bf16 gets fast weight
load; fp32r matmuls pay a ~300ns serialized weight load per matmul, measured
on HW). Scores are computed transposed (scoresT[s,t]) so the softmax
denominator falls out of the attn@v matmul via an appended ones-column in
its stationary operand (reduction over the PSUM partition dim); no
max-subtraction is needed (|scores| < ~3 by construction).

Stage C is a flat software pipeline over all 128 (chunk, s-tile) iterations:
scores+exp lead attn@v by LAG=2 iterations so the statically-ordered PE
stream never waits on the exp of its own iteration; per-chunk normalization
(fast reciprocal + partition-broadcast + multiply, on DVE/GpSimd) runs off
the PE path, and the projection of each chunk is emitted spread across the
next chunk's iterations. The second half of the QKV projection and of the
v-transposes is interleaved into the pipeline as PE filler work.
"""

import numpy as np

import concourse.mybir as mybir
from concourse import bacc
from concourse.bass_utils import run_bass_kernel_spmd
from concourse.masks import make_identity
from concourse.tile import TileContext

F32 = mybir.dt.float32
F32R = mybir.dt.float32r

B, T, D, H = 2, 2048, 1024, 16
N_CORES = 8
HPC = H // N_CORES          # heads per core (2)
DH = D // H                 # head dim (64)
BT = B * T                  # 4096 tokens
TC = 512                    # token chunk (psum free dim)
NDT = D // 128              # 8 d-tiles
NTC = BT // TC              # 8 global token chunks
EC = 3 * HPC * DH           # 384 local qkv rows
EL = HPC * DH               # 128 local e-dims (2 heads)
VBLK = EL + HPC             # 130: v block width with 2 ones columns

_NC_CACHE = {}


def _build_nc():
    nc = bacc.Bacc("TRN2", target_bir_lowering=False, debug=False,
                   num_devices=N_CORES)
    xT = nc.dram_tensor("xT", [D, BT], F32R, kind="ExternalInput").ap()
    wT = nc.dram_tensor("wT", [D, EC], F32R, kind="ExternalInput").ap()
    wpT = nc.dram_tensor("wpT", [EL, D], F32R, kind="ExternalInput").ap()
    y = nc.dram_tensor("y", [BT, D], F32, kind="ExternalOutput").ap()

    with TileContext(nc) as tc:
        with (
            tc.tile_pool(name="const", bufs=1) as constp,
            tc.tile_pool(name="big", bufs=1) as bigp,
            tc.tile_pool(name="xin", bufs=4) as xin,
            tc.tile_pool(name="at", bufs=6) as atp,
            tc.tile_pool(name="norm", bufs=2) as normp,
            tc.tile_pool(name="on", bufs=2) as onp,
            tc.tile_pool(name="yout", bufs=4) as yp,
            tc.tile_pool(name="sc", bufs=2, space="PSUM") as scp,
            tc.tile_pool(name="qkv", bufs=1, space="PSUM") as qkvp,
            tc.tile_pool(name="acc", bufs=3, space="PSUM") as accp,
        ):
            ident = constp.tile([128, 128], F32)
            make_identity(nc, ident[:])
            ones_src = constp.tile([128, 1], F32)
            nc.gpsimd.memset(ones_src[:], 1.0)

            w_sb = constp.tile([128, NDT, EC], F32R)
            nc.sync.dma_start(
                out=w_sb[:], in_=wT.rearrange("(n p) e -> p n e", p=128))
            wp_sb = constp.tile([128, D], F32R)
            nc.sync.dma_start(out=wp_sb[:], in_=wpT[:])

            q_sb = bigp.tile([128, BT], F32R, tag="q")
            k_sb = bigp.tile([128, BT], F32R, tag="k")
            v_sb = bigp.tile([128, BT], F32, tag="v")
            qkv_dst = [q_sb, k_sb, v_sb]
            vbuf = bigp.tile([128, (BT // 128) * VBLK], F32R, tag="vb")
            # static ones columns of the attn@v stationary operand:
            # columns 64, 129, 194, ... (stride 65 starting at 64)
            nc.vector.tensor_copy(
                vbuf[:, DH::DH + 1],
                ones_src[:].broadcast_to([128, (BT // 128) * HPC]))

            xT_r = xT.rearrange("(n p) t -> p n t", p=128)

            # ---- emission helpers ------------------------------------------
            def emit_qkv_chunk_dma(ci):
                x_t = xin.tile([128, NDT, TC], BF16, tag="x",
                               name=f"x_{ci}")
                h = NDT // 2
                nc.sync.dma_start(
                    out=x_t[:, 0:h, :],
                    in_=xT_r[:, 0:h, ci * TC:(ci + 1) * TC])
                nc.sync.dma_start(
                    out=x_t[:, h:NDT, :],
                    in_=xT_r[:, h:NDT, ci * TC:(ci + 1) * TC])
                return x_t

            def emit_qkv_e(ci, e, x_t, pool):
                ps = pool.tile([128, TC], F32, tag="qkv" if pool is qkvp
                               else "sc", name=f"qkvps_{ci}_{e}")
                for d in range(NDT):
                    nc.tensor.matmul(
                        ps[:], w_sb[:, d, e * EL:(e + 1) * EL],
                        x_t[:, d, :], start=(d == 0), stop=(d == NDT - 1))
                nc.vector.tensor_copy(
                    qkv_dst[e][:, ci * TC:(ci + 1) * TC], ps[:])

            def emit_vtrans(j):
                tp = accp.tile([128, 128], BF16, tag="acc",
                               name=f"tp_{j}")
                nc.tensor.transpose(
                    tp[:], v_sb[:, j * 128:(j + 1) * 128], ident[:])
                dst = vbuf[:, j * VBLK:(j + 1) * VBLK].rearrange(
                    "p (g e) -> p g e", g=HPC)[:, :, 0:DH]
                src_ = tp[:].rearrange("p (g e) -> p g e", g=HPC)
                nc.vector.tensor_copy(dst, src_)

            # ---- Stage C pipeline helpers ----------------------------------
            CHUNKS = [(b, tci) for b in range(B) for tci in range(T // TC)]
            NS = T // 128
            NI = len(CHUNKS) * NS
            LAG = 3
            PROJ_DELAY = 4
            acc_tiles = {}
            at_tiles = {}
            on_tiles = {}

            def emit_scores(ci, s):
                b, tci = CHUNKS[ci]
                g0 = b * T + tci * TC
                sg = b * T + s * 128
                at = atp.tile([128, HPC * TC], BF16, tag="at",
                              name=f"at_{ci}_{s}")
                sc = scp.tile([128, HPC * TC], F32, tag="sc",
                              name=f"sc_{ci}_{s}")
                for h in range(HPC):
                    r0 = h * DH
                    nc.tensor.matmul(
                        sc[:, h * TC:(h + 1) * TC],
                        k_sb[r0:r0 + DH, sg:sg + 128],
                        q_sb[r0:r0 + DH, g0:g0 + TC],
                        start=True, stop=True, tile_position=(r0, 0))
                nc.scalar.activation(
                    at[:], sc[:], mybir.ActivationFunctionType.Exp,
                    scale=float(1.0 / np.sqrt(DH)))
                at_tiles[(ci, s)] = at

            def emit_av(ci, s, h):
                b, tci = CHUNKS[ci]
                if s == 0 and h == 0:
                    acc_tiles[ci] = [
                        accp.tile([DH + 1, TC], F32, tag="acc",
                                  name=f"acc_{ci}_{hh}")
                        for hh in range(HPC)]
                at = at_tiles[(ci, s)]
                sg = b * T + s * 128
                blk = (sg // 128) * VBLK
                nc.tensor.matmul(
                    acc_tiles[ci][h][:],
                    vbuf[:, blk + h * (DH + 1):blk + (h + 1) * (DH + 1)],
                    at[:, h * TC:(h + 1) * TC],
                    start=(s == 0), stop=(s == NS - 1))
                if h == HPC - 1:
                    at_tiles.pop((ci, s))

            def emit_norm(ci):
                out_ps = acc_tiles.pop(ci)
                on = onp.tile([128, TC], BF16, tag="on", name=f"on_{ci}")
                for h in range(HPC):
                    den = normp.tile([1, TC], F32, tag="den",
                                     name=f"den_{ci}_{h}")
                    nc.vector.tensor_copy(den[:], out_ps[h][DH:DH + 1, :])
                    rc = normp.tile([1, TC], F32, tag="rc",
                                    name=f"rc_{ci}_{h}")
                    nc.vector.reciprocal_approx_fast(out=rc[:], in_=den[:])
                    bc = normp.tile([DH, TC], F32, tag="bc",
                                    name=f"bc_{ci}_{h}")
                    nc.gpsimd.partition_broadcast(bc[:], rc[:])
                    nc.vector.tensor_mul(
                        on[h * DH:(h + 1) * DH, :],
                        out_ps[h][0:DH, :], bc[:])
                on_tiles[ci] = on

            def emit_proj_tt(ci, tt):
                b, tci = CHUNKS[ci]
                g0 = b * T + tci * TC
                on = on_tiles[ci]
                yps = scp.tile([128, D], F32, tag="sc",
                               name=f"yps_{ci}_{tt}")
                for dc in range(D // 512):
                    nc.tensor.matmul(
                        yps[:, dc * 512:(dc + 1) * 512],
                        on[:, tt * 128:(tt + 1) * 128],
                        wp_sb[:, dc * 512:(dc + 1) * 512],
                        start=True, stop=True)
                ys = yp.tile([128, D], F32, tag="y", name=f"ys_{ci}_{tt}")
                nc.vector.tensor_copy(ys[:], yps[:])
                nc.sync.dma_start(
                    out=y[g0 + tt * 128:g0 + (tt + 1) * 128, :],
                    in_=ys[:])
                if tt == TC // 128 - 1:
                    on_tiles.pop(ci)

            # ---- Prologue: QKV chunks 0..1 + v-transpose blocks 0..7 -------
            # DMA order: first x half, then weights (first matmul needs both;
            # x is issued first so transfers overlap the w trigger), wp last.
            h2 = NDT // 2
            for ci in range(2):
                if ci == 0:
                    x_t = xin.tile([128, NDT, TC], BF16, tag="x", name="x_0")
                    nc.sync.dma_start(out=x_t[:, 0:h2, :],
                                      in_=xT[:, 0, 0:h2, :])
                    nc.sync.dma_start(out=w_sb[:, 0:h2, :],
                                      in_=wT[:, 0:h2, :])
                    nc.sync.dma_start(out=x_t[:, h2:NDT, :],
                                      in_=xT[:, 0, h2:NDT, :])
                    nc.sync.dma_start(out=w_sb[:, h2:NDT, :],
                                      in_=wT[:, h2:NDT, :])
                else:
                    x_t = emit_qkv_chunk_dma(ci)
                for e in range(3):
                    emit_qkv_e(ci, e, x_t, scp)
                if ci == 1:
                    nc.sync.dma_start(out=wp_sb[:], in_=wpT[:])
            for j in range(8):
                emit_vtrans(j)

            # ---- Filler work, deadline-ordered ------------------------------
            # cadence 1 for the first 16 pipeline iterations (feeds the rest
            # of batch 0), cadence 2 afterwards (feeds batch 1 by idx ~78).
            fillers = []
            for ci, blocks in ((2, range(8, 12)), (3, range(12, 16))):
                fillers.append(("dma", ci))
                for e in range(3):
                    fillers.append(("qkv", ci, e))
                for j in blocks:
                    fillers.append(("vt", j))
            for ci in range(4, NTC):
                fillers.append(("dma", ci))
                for e in range(3):
                    fillers.append(("qkv", ci, e))
                for j in range(4 * ci, 4 * ci + 4):
                    fillers.append(("vt", j))
            filler_x = {}

            def emit_filler():
                if not fillers:
                    return
                f = fillers.pop(0)
                if f[0] == "dma":
                    filler_x[f[1]] = emit_qkv_chunk_dma(f[1])
                elif f[0] == "qkv":
                    emit_qkv_e(f[1], f[2], filler_x[f[1]], qkvp)
                else:
                    emit_vtrans(f[1])

            # ---- Main pipeline ---------------------------------------------
            for idx in range(NI + LAG + 4 + 2 * (TC // 128) + 1):
                if idx < NI:
                    ci, s = divmod(idx, NS)
                    emit_scores(ci, s)
                if idx < 16 or idx % 2 == 0:
                    emit_filler()
                j = idx - LAG
                if 0 <= j < NI:
                    cj, sj = divmod(j, NS)
                    emit_av(cj, sj, 0)
                j2 = idx - LAG - 1
                if 0 <= j2 < NI:
                    cj2, sj2 = divmod(j2, NS)
                    emit_av(cj2, sj2, 1)
                    if sj2 == NS - 1:
                        emit_norm(cj2)
                for tt in range(TC // 128):
                    k = idx - LAG - 3 - 2 * tt
                    if 0 <= k and (k % NS) == NS - 1 and k // NS < len(CHUNKS):
                        emit_proj_tt(k // NS, tt)
            while fillers:
                emit_filler()

    nc.finalize()
    return nc


def _get_nc():
    if "nc" not in _NC_CACHE:
        _NC_CACHE["nc"] = _build_nc()
    return _NC_CACHE["nc"]


def kernel(x, w_qkv, w_proj):
    x = np.ascontiguousarray(x, dtype=np.float32)
    w_qkv = np.ascontiguousarray(w_qkv, dtype=np.float32)
    w_proj = np.ascontiguousarray(w_proj, dtype=np.float32)

    xT = np.ascontiguousarray(x.reshape(BT, D).T)
    in_maps = []
    for c in range(N_CORES):
        rows = slice(c * HPC * DH, (c + 1) * HPC * DH)
        w_c = np.concatenate(
            [w_qkv[0 * D:, :][rows], w_qkv[1 * D:, :][rows],
             w_qkv[2 * D:, :][rows]], axis=0)            # [384, 1024]
        wT_c = np.ascontiguousarray(w_c.T)               # [1024, 384]
        wpT_c = np.ascontiguousarray(w_proj[:, rows].T)  # [128, 1024]
        in_maps.append({"xT": xT, "wT": wT_c, "wpT": wpT_c})

    nc = _get_nc()
    res = None
    for attempt in range(3):
        try:
            res = run_bass_kernel_spmd(nc, in_maps,
                                       core_ids=list(range(N_CORES)))
            break
        except Exception:
            # transient device errors (e.g. NRT_EXEC_UNIT_UNRECOVERABLE)
            # have been observed to succeed on retry
            if attempt == 2:
                raise
    y = res.results[0]["y"].astype(np.float32)
    for c in range(1, N_CORES):
        y = y + res.results[c]["y"]
    return y.reshape(B, T, D)
